# revision 33
# baseline (speedup 1.0000x reference)
"""Trainium2 Bass kernel: causal self-attention with RoPE.

Problem: B=2, T=2048, C=1536, H=16 heads, D=96 head dim.
  qkv = x @ w_attn.T ; rope(q, k) ; causal softmax attention ; y = att @ w_proj.T

Sharding (8 cores): data-parallel over batch (2) x tensor-parallel over heads
(4 groups of 4 heads).  Each core computes, for its batch b and its 4 heads:
QKV projection, RoPE + causal attention, and its partial output projection
y_part = att_heads @ w_proj[:, cols].T; the 4 partials per batch are summed on
the host.

This version is a single merged pipeline (no phase barrier): for each
512-token quarter q we emit QKV+rope+transpose for that quarter, then the
attention for q-tile q (which only needs K/V up to quarter q), then the output
projection for q-tile q.  The Tile scheduler's per-engine ready heaps then
overlap attention's scalar/vector work (exp, normalize) with the next
quarter's QKV matmuls, keeping the PE continuously busy (no HAM re-throttle).

All matmul operands are bf16 (PSUM accumulation stays fp32).  The weights
carry 1/sqrt(C) scaling, so bf16 input rounding adds only ~0.2% noise to the
logits -- far inside the harness' 2e-2 gate.  bf16 also halves HBM traffic
and SBUF footprint, letting every pool stay open for the whole kernel.

PSUM budget (8 banks): qkv accumulators 3, transpose staging 1, S-block pair
2, PV accumulator 1, projection 1.
"""

import math

import numpy as np

import concourse.bass as bass
import concourse.mybir as mybir
import concourse.tile as tile
from concourse import bacc, bass_utils
from concourse.masks import make_identity

# ---------------------------------------------------------------- constants
B, T, C = 2, 2048, 1536
H, D = 16, 96
NCORES = 8
HPC = 4                      # heads per core
DH = HPC * D                 # 384 = per-core head-dim total
DH2 = HPC * (D // 2)         # 192 = per-core evens (or odds) width
SCALE = 1.0 / math.sqrt(D)
NT = T // 128                # 16 t-tiles of 128 tokens
NQ = T // 512                # 4 q-tiles of 512 queries
F32 = mybir.dt.float32
F32R = mybir.dt.float32r
BF16 = mybir.dt.bfloat16


def _qe(j, q0, qw=512):
    """Causally-live query start for key block j within the query window
    of width qw starting at absolute query q0."""
    return min(max(j * 128 - q0, 0), qw - 128)


# ---------------------------------------------------------------- device IR
def _build_kernel(reps=1):
    nc = bacc.Bacc(
        "TRN2",
        target_bir_lowering=False,
        debug=False,
        enable_asserts=False,
        num_devices=NCORES,
    )

    xT = nc.dram_tensor("xT", [C, T], BF16, kind="ExternalInput").ap()
    wqkvT = nc.dram_tensor("wqkvT", [C, 3 * DH], BF16, kind="ExternalInput").ap()
    wpT = nc.dram_tensor("wpT", [128, 3, C], BF16, kind="ExternalInput").ap()
    tab3 = nc.dram_tensor("tab3", [T, 3 * DH2], BF16, kind="ExternalInput").ap()
    tmd = nc.dram_tensor("tm", [128, 1024], BF16, kind="ExternalInput").ap()
    yp = nc.dram_tensor("yp", [T, C], BF16, kind="ExternalOutput").ap()

    with tile.TileContext(nc) as tc:
        for _ in range(reps):
            _body(tc, xT, wqkvT, wpT, tab3, tmd, yp)

    nc.compile()
    return nc


def _body(tc, xT, wqkvT, wpT, tab3, tmd, yp):
    nc = tc.nc
    Exp = mybir.ActivationFunctionType.Exp

    with (
        tc.tile_pool(name="persist", bufs=1) as persist,
        tc.tile_pool(name="pax", bufs=4) as pax,
        tc.tile_pool(name="ptab", bufs=2) as ptab,
        tc.tile_pool(name="pqt", bufs=2) as pqt,
        tc.tile_pool(name="par", bufs=1) as par,
        tc.tile_pool(name="prk", bufs=2) as prk,
        tc.tile_pool(name="ppt", bufs=5) as ppt,
        tc.tile_pool(name="pat", bufs=4) as pat,
        tc.tile_pool(name="pys", bufs=3) as pys,
        tc.tile_pool(name="pbr", bufs=2) as pbr,
        tc.tile_pool(name="pst", bufs=1, space="PSUM") as pst,
        tc.tile_pool(name="pacc", bufs=1, space="PSUM") as pacc,
        tc.tile_pool(name="pop", bufs=1, space="PSUM") as pop,
    ):
        # ---------------- persistent tiles --------------------------------
        KT = persist.tile([D, HPC, T], BF16)          # rope'd K^T
        V = persist.tile([128, HPC, NT, D + 1], BF16)  # V + ones col (denom)
        tm = persist.tile([128, 1024], BF16)           # causal mask
        wqA = persist.tile([128, 6, 3 * DH], BF16)
        wqB = persist.tile([128, 6, 3 * DH], BF16)
        wqH = (wqA, wqB)
        wp_sb = persist.tile([128, 3, C], BF16)
        ident = persist.tile([128, 128], BF16)
        identf = persist.tile([128, 128], F32)
        onesf = persist.tile([128, D], F32)
        ones1 = persist.tile([1, D], F32R)

        # ---------------- setup + startup DMA -----------------------------
        # Sync ring: tab3 quarter 0, then x quarter 0 (interleaved with the
        # weight tiles on the scalar ring so the first matmul can start after
        # ~one tile of each).
        t3s = [None] * 4
        t3s[0] = ptab.tile([128, 4, 3 * DH2], BF16, name="t3_0", tag="t3")
        nc.sync.dma_start(
            out=t3s[0],
            in_=tab3[0:512, :].rearrange("(tt p) d -> p tt d", p=128),
        )
        # Coalesced startup loads: the weight stream in 4 pieces (coarse
        # trickle) and quarter-0 x in 3, alternating rings -- ~10 DMA
        # issues instead of 26, halving engine-queue issue time
        xqs = [None] * 4
        xq0a = pax.tile([128, 6, 512], BF16, name="xq0a", tag="xq")
        xq0b = pax.tile([128, 6, 512], BF16, name="xq0b", tag="xq")
        xqs[0] = (xq0a, xq0b)
        for p in range(2):
            c0, c1 = p * 6, p * 6 + 6
            weng = nc.scalar if p == 0 else nc.sync
            xeng = nc.sync if p == 0 else nc.scalar
            xeng.dma_start(
                out=xqs[0][p],
                in_=xT[c0 * 128 : c1 * 128, 0:512].rearrange(
                    "(c p) t -> p c t", p=128
                ),
            )
            weng.dma_start(
                out=wqH[p],
                in_=wqkvT[c0 * 128 : c1 * 128, :].rearrange(
                    "(c p) d -> p c d", p=128
                ),
            )
        nc.scalar.dma_start(out=wp_sb, in_=wpT)
        nc.sync.dma_start(out=tm, in_=tmd)
        make_identity(nc, identf)
        nc.scalar.copy(out=ident, in_=identf)
        # HAM warm-up: ~4us of dependency-free matmuls on the identity so
        # the PE clock is already 8/8 when the first weights land
        wu = pop.tile([128, 512], F32, name="wu", tag="op")

        def keepwarm(n, dst=None, w=4):
            dst = wu if dst is None else dst
            for r in range(n):
                nc.tensor.matmul(
                    dst[:, (r % w) * 128 : (r % w + 1) * 128],
                    ident,
                    ident,
                    start=True,
                    stop=True,
                )

        keepwarm(32)
        nc.vector.memset(onesf, 1.0)
        nc.scalar.copy(
            out=V[:, :, :, D],
            in_=onesf[:, 0 : HPC * NT].rearrange("p (h t) -> p h t", h=HPC),
        )
        nc.scalar.copy(out=ones1, in_=onesf[0:1, :])

        QTs = [None] * 4
        attTs = [None] * 4

        def emit_prefetch(qn):
            t3s[qn] = ptab.tile(
                [128, 4, 3 * DH2], BF16, name=f"t3_{qn}", tag="t3"
            )
            nc.sync.dma_start(
                out=t3s[qn],
                in_=tab3[qn * 512 : (qn + 1) * 512, :].rearrange(
                    "(tt p) d -> p tt d", p=128
                ),
            )
            xqa = pax.tile([128, 6, 512], BF16, name=f"xq{qn}a", tag="xq")
            xqb = pax.tile([128, 6, 512], BF16, name=f"xq{qn}b", tag="xq")
            for p, xq in enumerate((xqa, xqb)):
                nc.sync.dma_start(
                    out=xq,
                    in_=xT[
                        p * 768 : (p + 1) * 768, qn * 512 : (qn + 1) * 512
                    ].rearrange("(c p) t -> p c t", p=128),
                )
            xqs[qn] = (xqa, xqb)

        def emit_quarter(q, ppq, ptp):
            if q < 3:
                emit_prefetch(q + 1)
            QT = pqt.tile([D, HPC, 512], BF16, name=f"QT{q}", tag="QT")
            QTs[q] = QT
            t3 = t3s[q]
            for tt in range(4):
                t0 = q * 4 + tt  # global 128-token tile index
                qp = ppq.tile([128, DH], F32, tag="qp")
                kp = ppq.tile([128, DH], F32, tag="kp")
                vp = ppq.tile([128, DH], F32, tag="vp")
                for c in range(12):
                    lhs = xqs[q][c // 6][:, c % 6, tt * 128 : (tt + 1) * 128]
                    w = wqH[c // 6][:, c % 6, :]
                    s0 = c == 0
                    s1 = c == 11
                    nc.tensor.matmul(qp, lhs, w[:, 0:DH], start=s0, stop=s1)
                    nc.tensor.matmul(
                        kp, lhs, w[:, DH : 2 * DH], start=s0, stop=s1
                    )
                    nc.tensor.matmul(
                        vp, lhs, w[:, 2 * DH : 3 * DH], start=s0, stop=s1
                    )
                    if q == 0 and tt == 0:
                        # keep the PE clock warm while the weight stream
                        # trickles in during the first accumulation
                        keepwarm(4)

                # V: [t, (h d)] -> V[:, h, t0, 0:D]
                nc.scalar.copy(
                    out=V[:, :, t0, 0:D],
                    in_=vp.rearrange("p (h d) -> p h d", h=HPC),
                )

                # rope: dst_even = e*c - o*s ; dst_odd = e*s + o*c
                # t3 layout: [s | c | -s]: [c|-s] = t3[192:576], [s|c] =
                # t3[0:384].  Muls on Vector, adds on GpSimd.
                qr = prk.tile([128, HPC, 2, D // 2], BF16, tag="qr")
                kr = prk.tile([128, HPC, 2, D // 2], BF16, tag="kr")
                for (src, dst, tag) in ((qp, qr, "q"), (kp, kr, "k")):
                    t12 = par.tile([128, DH], F32, tag=f"t12{tag}")
                    t34 = par.tile([128, DH], F32, tag=f"t34{tag}")
                    nc.vector.tensor_mul(t12, src, t3[:, tt, DH2 : DH2 + DH])
                    nc.vector.tensor_mul(t34, src, t3[:, tt, 0:DH])
                    nc.gpsimd.tensor_add(
                        dst[:, :, 0, :],
                        t12[:, 0:DH2].rearrange("p (h d) -> p h d", h=HPC),
                        t12[:, DH2:DH].rearrange("p (h d) -> p h d", h=HPC),
                    )
                    nc.gpsimd.tensor_add(
                        dst[:, :, 1, :],
                        t34[:, 0:DH2].rearrange("p (h d) -> p h d", h=HPC),
                        t34[:, DH2:DH].rearrange("p (h d) -> p h d", h=HPC),
                    )

                # transpose rope'd q/k tiles through one PSUM bank
                tpq = ptp.tile([D, HPC, 128], BF16, tag="tp")
                for h in range(HPC):
                    nc.tensor.transpose(tpq[:, h], qr[:, h], ident)
                if q == 3:
                    # ACT is backlogged with attn(2) exps here; do not let
                    # the copy that gates attn(3) queue behind them
                    nc.vector.tensor_copy(
                        QT[:, :, tt * 128 : (tt + 1) * 128], tpq
                    )
                else:
                    nc.scalar.copy(
                        out=QT[:, :, tt * 128 : (tt + 1) * 128], in_=tpq
                    )
                tpk = ptp.tile([D, HPC, 128], BF16, tag="tp")
                for h in range(HPC):
                    nc.tensor.transpose(tpk[:, h], kr[:, h], ident)
                nc.vector.tensor_copy(
                    KT[:, :, t0 * 128 : (t0 + 1) * 128], tpk
                )

        def emit_attn(i, stps, accs, lag=1, hgroup=1, den_vec=False):
            # q-tile i.  Software-pipelined: the PV pair for jp trails its S
            # pair by `lag` pairs, so in the PE's in-order stream the next S
            # pairs sit BEFORE a PV that is still waiting on exp.  With
            # hgroup=2, two heads' pair-streams interleave so one head's
            # normalize chain hides behind the other head's exp stream.
            q0 = i * 512
            QT = QTs[i]
            # attT in flat-(h,d) layout: row h*96+d -> [chunk r//128, r%128]
            attT = pat.tile([128, 3, 512], BF16, name=f"attT{i}", tag="attT")
            attTs[i] = attT
            nblk = 4 * i + 4
            jmax = nblk - 1
            npairs = nblk // 2
            pts = {}
            accd = {}

            def emit_s_exp(h, jp, pool):
                j0 = 2 * jp
                stp = pool.tile([128, 2, 512], F32, tag="stp")
                for jj in range(2):
                    j = j0 + jj
                    qe = _qe(j, q0)
                    nc.tensor.matmul(
                        stp[:, jj, qe:],
                        KT[:, h, j * 128 : (j + 1) * 128],
                        QT[:, h, qe:512],
                        start=True,
                        stop=True,
                    )
                pt = ppt.tile([128, 2, 512], BF16, tag="pt")
                pts[(h, jp)] = pt
                if _qe(j0, q0) > 0 or _qe(j0 + 1, q0) > 0:
                    # trimmed diagonal blocks: exp over the live range only
                    for jj in range(2):
                        qe = _qe(j0 + jj, q0)
                        nc.scalar.activation(
                            pt[:, jj, qe:], stp[:, jj, qe:], Exp, scale=SCALE
                        )
                else:
                    nc.scalar.activation(
                        pt.rearrange("p a b -> p (a b)"),
                        stp.rearrange("p a b -> p (a b)"),
                        Exp,
                        scale=SCALE,
                    )
                for jj in range(2):
                    j = j0 + jj
                    off = j * 128 - q0
                    if off >= 0:  # diagonal block: causal mask
                        qs = _qe(j, q0)
                        qf = off + 128
                        nc.gpsimd.tensor_mul(
                            pt[:, jj, qs:qf],
                            pt[:, jj, qs:qf],
                            tm[:, 512 - off + qs : 512 - off + qf],
                        )

            def emit_pv(h, jp):
                j0 = 2 * jp
                pt = pts.pop((h, jp))
                acc = accd[h]
                for jj in range(2):
                    j = j0 + jj
                    qe = _qe(j, q0)
                    nc.tensor.matmul(
                        acc[:, qe:],
                        V[:, h, j],
                        pt[:, jj, qe:],
                        start=(j == 0),
                        stop=(j == jmax),
                    )

            def normalize(h):
                # attT[:, h] = acc[0:D] * (1 / acc[D]) per column
                acc = accd[h]
                denS = pbr.tile([1, 512], F32, tag="denS")
                if den_vec:
                    nc.vector.tensor_copy(denS, acc[D : D + 1, :])
                else:
                    nc.scalar.copy(out=denS, in_=acc[D : D + 1, :])
                r1 = pbr.tile([1, 512], F32, tag="r1")
                nc.vector.reciprocal_approx_fast(r1, denS)
                r1r = pbr.tile([1, 512], F32R, tag="r1r")
                nc.vector.tensor_copy(r1r, r1)
                # broadcast 1/den across partitions via a K=1 matmul into a
                # reused stp slot
                rept = stps[0].tile([128, 2, 512], F32, tag="stp")
                rep = rept[0:D, 0, :]
                nc.tensor.matmul(rep, ones1, r1r, start=True, stop=True)
                reps_t = pbr.tile([D, 512], F32, tag="reps")
                nc.vector.tensor_copy(reps_t, rep)
                # scatter normalized rows into the flat-(h,d) layout, split
                # so each piece obeys the partition-alignment rule (a
                # pattern starting at partition 32/96 spans <= 32)
                def _allowed(b):
                    return 128 - b if b % 64 == 0 else 64 - b % 64

                r0 = h * D
                d0 = 0
                while d0 < D:
                    ch, row = (r0 + d0) // 128, (r0 + d0) % 128
                    dn = min(
                        D - d0, 128 - row, _allowed(d0 % 128), _allowed(row)
                    )
                    nc.vector.tensor_mul(
                        attT[row : row + dn, ch, :],
                        acc[d0 : d0 + dn, :],
                        reps_t[d0 : d0 + dn, :],
                    )
                    d0 += dn

            for hbase in range(0, HPC, hgroup):
                hs = list(range(hbase, hbase + hgroup))
                for k, h in enumerate(hs):
                    accd[h] = accs[k % len(accs)].tile(
                        [D + 1, 512], F32, name=f"acc{i}_{h}", tag="acc"
                    )
                for jp in range(npairs + lag):
                    for k, h in enumerate(hs):
                        if jp < npairs:
                            emit_s_exp(h, jp, stps[k % len(stps)])
                        if jp >= lag:
                            emit_pv(h, jp - lag)
                for h in hs:
                    normalize(h)
                if i == 3:
                    kwt = stps[0].tile(
                        [128, 2, 512], F32, name="kwt", tag="stp"
                    )
                    keepwarm(8, kwt[:, 0, :])

        def emit_proj(i, pops=None, tts=range(4), mix_copy=False,
                      dma_split=False, warm=False):
            pops = pops or [pop]
            q0 = i * 512
            attT = attTs[i]
            for tt in tts:
                r0 = q0 + tt * 128
                ysb = pys.tile([128, C], BF16, tag="ysb")
                for os in range(3):
                    op = pops[(tt * 3 + os) % len(pops)].tile(
                        [128, 512], F32, tag="op"
                    )
                    for ch in range(3):
                        nc.tensor.matmul(
                            op,
                            attT[:, ch, tt * 128 : (tt + 1) * 128],
                            wp_sb[:, ch, os * 512 : (os + 1) * 512],
                            start=(ch == 0),
                            stop=(ch == 2),
                        )
                    if mix_copy and os < 2:
                        nc.scalar.copy(
                            out=ysb[:, os * 512 : (os + 1) * 512], in_=op
                        )
                    else:
                        nc.vector.tensor_copy(
                            ysb[:, os * 512 : (os + 1) * 512], op
                        )
                if dma_split:
                    for os in range(3):
                        nc.sync.dma_start(
                            out=yp[r0 : r0 + 128, os * 512 : (os + 1) * 512],
                            in_=ysb[:, os * 512 : (os + 1) * 512],
                        )
                else:
                    nc.sync.dma_start(out=yp[r0 : r0 + 128, :], in_=ysb)
                if warm:
                    # junk matmuls between tile groups: the PE idles here
                    # waiting on copies, which tips HAM into re-throttle
                    kw = pops[tt % len(pops)].tile(
                        [128, 512], F32, name=f"kwp{i}_{tt}", tag="op"
                    )
                    keepwarm(4, kw)

        # ---------------- emission sequence -------------------------------
        # proj(i) is emitted after attn(i+1) so projection matmuls serve as
        # PE filler during the following attention's exp stalls; attn(3)
        # gets double-buffered S/acc PSUM from the banks the QKV
        # accumulators free after quarter 3.
        with (
            tc.tile_pool(name="ppq", bufs=1, space="PSUM") as ppq,
            tc.tile_pool(name="ptp", bufs=1, space="PSUM") as ptp,
        ):
            emit_quarter(0, ppq, ptp)
            emit_attn(0, [pst], [pacc], lag=1)
            emit_quarter(1, ppq, ptp)
            emit_attn(1, [pst], [pacc], lag=1)
            emit_proj(0, tts=(0, 1))
            emit_quarter(2, ppq, ptp)
            emit_attn(2, [pst], [pacc], lag=1)
            emit_proj(1, tts=(0, 1))
            emit_quarter(3, ppq, ptp)
            # bridge the transition into attn(3): dependency-free matmuls
            # run while attn(3)'s first S waits on the final QT copy, so
            # the PE never idles past the HAM re-throttle window
            kwq = ppq.tile([128, DH], F32, name="kwq", tag="qp")
            keepwarm(16, kwq, w=3)
        with (
            tc.tile_pool(name="pstB", bufs=1, space="PSUM") as pstB,
            tc.tile_pool(name="paccB", bufs=1, space="PSUM") as paccB,
            tc.tile_pool(name="popB", bufs=1, space="PSUM") as popB,
        ):
            emit_proj(0, [pop, popB], tts=(2, 3))
            # bridge the transition with independent work: proj(2) tile 0
            # runs while attn(3)'s first S waits on the final QT copy
            emit_proj(2, [pop, popB], tts=(0,))
            emit_attn(3, [pst, pstB], [pacc, paccB], lag=1, hgroup=2,
                      den_vec=True)
            emit_proj(2, [pop, popB], tts=(1, 2, 3), warm=True)
            emit_proj(1, [pop, popB], tts=(2, 3), mix_copy=True, warm=True)
            emit_proj(3, [pop, popB], mix_copy=True, dma_split=True,
                      warm=True)


# ---------------------------------------------------------------- host side
def _rope_tables():
    inv_freq = 1.0 / (10000.0 ** (np.arange(0, D, 2, dtype=np.float32) / D))
    t = np.arange(T, dtype=np.float32)
    freqs = np.outer(t, inv_freq)                       # [T, 48]
    emb = np.concatenate([freqs, freqs], axis=-1)       # [T, 96]
    c = np.cos(emb)[:, ::2].astype(np.float32)          # [T, 48]
    s = np.sin(emb)[:, ::2].astype(np.float32)
    ct = np.ascontiguousarray(np.tile(c, (1, HPC)))     # [T, 192]
    st = np.ascontiguousarray(np.tile(s, (1, HPC)))
    # [s | c | -s]: [c|-s] = tab3[:, 192:576], [s|c] = tab3[:, 0:384]
    tab3 = np.ascontiguousarray(np.concatenate([st, ct, -st], axis=1))
    return tab3.astype(mybir.dt.np(BF16))


def _tri_mask():
    # tm[k, c] = 1.0 iff c >= k + 512
    k = np.arange(128)[:, None]
    c = np.arange(1024)[None, :]
    return (c >= k + 512).astype(mybir.dt.np(BF16))


def _core_inputs(x, w_attn, w_proj, core):
    b, g = divmod(core, HPC)
    heads = [HPC * g + hh for hh in range(HPC)]
    bf = mybir.dt.np(BF16)
    xTh = np.ascontiguousarray(x[b].T).astype(bf)       # [C, T]

    def rows(sec, h):
        return w_attn[sec * C + h * D : sec * C + (h + 1) * D]

    q_e = np.concatenate([rows(0, h)[0::2] for h in heads])   # [192, C]
    q_o = np.concatenate([rows(0, h)[1::2] for h in heads])
    k_e = np.concatenate([rows(1, h)[0::2] for h in heads])
    k_o = np.concatenate([rows(1, h)[1::2] for h in heads])
    v_r = np.concatenate([rows(2, h) for h in heads])         # [384, C]
    wqkv = np.concatenate([q_e, q_o, k_e, k_o, v_r])          # [1152, C]
    wqkvT = np.ascontiguousarray(wqkv.T).astype(bf)           # [C, 1152]

    wp_flat = np.concatenate(
        [w_proj[:, h * D : (h + 1) * D].T for h in heads]
    )                                                         # [384, C], (h,d)-major
    wpT = np.ascontiguousarray(
        wp_flat.reshape(3, 128, C).transpose(1, 0, 2)
    ).astype(bf)                                              # [128, 3, C]
    return {"xT": xTh, "wqkvT": wqkvT, "wpT": wpT}


_NC_CACHE = {}


def _get_nc(reps=1):
    if reps not in _NC_CACHE:
        _NC_CACHE[reps] = _build_kernel(reps)
    return _NC_CACHE[reps]


def make_in_maps(x, w_attn, w_proj):
    x = np.asarray(x, np.float32)
    w_attn = np.asarray(w_attn, np.float32)
    w_proj = np.asarray(w_proj, np.float32)
    tab3 = _rope_tables()
    tm = _tri_mask()
    in_maps = []
    for core in range(NCORES):
        m = _core_inputs(x, w_attn, w_proj, core)
        m["tab3"] = tab3
        m["tm"] = tm
        in_maps.append(m)
    return in_maps


def combine_outputs(results):
    y = np.empty((B, T, C), np.float32)
    for b in range(B):
        parts = [
            results[b * HPC + g]["yp"].astype(np.float32) for g in range(HPC)
        ]
        y[b] = parts[0] + parts[1] + parts[2] + parts[3]
    return y


def kernel(x, w_attn, w_proj, _trace=False, _trace_kwargs=None):
    nc = _get_nc()
    in_maps = make_in_maps(x, w_attn, w_proj)
    res = bass_utils.run_bass_kernel_spmd(
        nc,
        in_maps,
        core_ids=list(range(NCORES)),
        trace=_trace,
        **(_trace_kwargs or {}),
    )
    out = combine_outputs(res.results)
    if _trace:
        kernel._last_results = res
    return out
